# revision 81
# baseline (speedup 1.0000x reference)
"""Trainium2 Bass kernel for nn_ConditionalFeedForward (MoE top-2 routing).

Strategy: expert-parallel across 8 NeuronCores with a load-balancing "side
block". Core e owns expert e's weights and computes the first CAP_M routed
tokens of expert e; the overflow tokens of heavy experts are distributed as
<=CAP_S-token side blocks to other cores (each core carries one side block
with its own small weight stream). All matmul operands are fp16 (full
1 cycle/row PE rate, half the HBM traffic of fp32r); PSUM accumulates fp32.

Single fused pass per core, hT kept resident in SBUF (no DRAM staging):

    hT = silu(w1 @ xT) * (w3 @ xT)     # [FFN, CAP] fp16 slab in SBUF
    yT = w2 @ hT                       # [DIM, CAP] -> fp16 out

Host gathers/pads tokens per expert, pre-transposes weights into PE layouts,
and scatter-adds gate-weighted outputs back to the full [N_TOKENS, DIM]
result in fp32.
"""

import os
import numpy as np

import concourse.bacc as bacc
import concourse.mybir as mybir
import concourse.tile as tile
from concourse.bass_utils import run_bass_kernel_spmd

# Problem constants (hardcoded per harness contract)
NUM_EXPERTS = 8
DIM = 2048
FFN = 5632
N_CORES = 8
KD = DIM // 128    # 16 contraction chunks for GEMM1/3; output chunks GEMM2
KF = FFN // 128    # 44 ffn chunks

F32 = mybir.dt.float32
F16 = mybir.dt.float16

# Compiled program cache keyed by (cap_m, cap_s)
_PROGRAMS = {}

# Filled by the last kernel() call when BASS_KERNEL_TRACE=1 (for test.py)
LAST_EXEC_NS = None


def _tiles(total, mx=512):
    """Token tiles of <=512 (PSUM bank = 512 fp32)."""
    return [(t0, min(mx, total - t0)) for t0 in range(0, total, mx)]


def _xblocks(cap):
    """Column blocks for the x load: small leading blocks so the first PSUM
    groups start early, each a contiguous DRAM tensor (128 descriptors)."""
    if cap > 512:
        return [(0, 256), (256, 256)] + [(512 + t0, tn)
                                         for t0, tn in _tiles(cap - 512)]
    return _tiles(cap, 256)


def _build_program(cap_m, cap_s):
    nc = bacc.Bacc("TRN2", target_bir_lowering=False, debug=False,
                   num_devices=N_CORES)

    xblocks = _xblocks(cap_m)
    xm_d = [nc.dram_tensor(f"xm{j}", [128, KD, bw], F16, kind="ExternalInput")
            for j, (b0, bw) in enumerate(xblocks)]
    w1m_d = nc.dram_tensor("w1m", [KF, 128, KD, 128], F16, kind="ExternalInput")
    w3m_d = nc.dram_tensor("w3m", [KF, 128, KD, 128], F16, kind="ExternalInput")
    w2m_d = nc.dram_tensor("w2m", [KD, 128, KF, 128], F16, kind="ExternalInput")
    ym_d = nc.dram_tensor("ym", [KD, 128, cap_m], F16, kind="ExternalOutput")
    if cap_s:
        xs_d = nc.dram_tensor("xs", [128, KD, cap_s], F16, kind="ExternalInput")
        w1s_d = nc.dram_tensor("w1s", [KF, 128, KD, 128], F16, kind="ExternalInput")
        w3s_d = nc.dram_tensor("w3s", [KF, 128, KD, 128], F16, kind="ExternalInput")
        w2s_d = nc.dram_tensor("w2s", [KD, 128, KF, 128], F16, kind="ExternalInput")
        ys_d = nc.dram_tensor("ys", [KD, 128, cap_s], F16, kind="ExternalOutput")

    silu = mybir.ActivationFunctionType.Silu
    tiles_m = _tiles(cap_m)

    with tile.TileContext(nc) as tc:
        with (
            tc.tile_pool(name="x", bufs=1) as xpool,
            tc.tile_pool(name="h", bufs=1) as hpool,
            tc.tile_pool(name="w2h", bufs=1) as w2hpool,
            # one PSUM pool spans both phases (no pool-transition barrier
            # at the GEMM2 boundary): h1p/h3p 3 bufs + yp 2 = 8 banks
            tc.tile_pool(name="ps", bufs=3, space="PSUM") as psum1,
        ):
            xb_s = [xpool.tile([128, KD, bw], F16, name=f"xb{j}")
                    for j, (b0, bw) in enumerate(xblocks)]
            warm_s = xpool.tile([128, 640], F16)  # scratch for warm-up
            hm_s = hpool.tile([128, KF, cap_m], F16)
            if cap_s:
                xs_s = xpool.tile([128, KD, cap_s], F16)
                hs_s = hpool.tile([128, KF, cap_s], F16)
            # x loads on the ACT HWDGE ring (contiguous per-block DMAs; the
            # SP ring carries the weight stream), first-needed first: the
            # side block computes first, so its tiny x lands first. Blocks
            # >=2 are issued inside the f-loop so the early f's weight
            # chunks aren't queued behind them on the shared DMA engines.
            if cap_s:
                nc.scalar.dma_start(xs_s[:], xs_d[:])
            for j in range(len(xblocks)):
                nc.scalar.dma_start(xb_s[j][:], xm_d[j][:])
            w2m0 = w2hpool.tile([128, KF, 128], F16)
            if cap_s:
                w2s0 = w2hpool.tile([128, KF, 128], F16)

            # ---- Phase 1: hT = silu(w1 @ xT) * (w3 @ xT), SBUF-resident ----
            with (
                tc.tile_pool(name="w13", bufs=3) as wpool,
                tc.tile_pool(name="act", bufs=3) as spool,
            ):
                # Warm-up: a dummy matmul chain gated only on a cheap DVE
                # memset keeps the PE busy until the side block's operands
                # land (~4.3us), so the p-state ramp (0.65/1.2 GHz for the
                # first 3us of activity) is spent on throwaway work and
                # real matmuls start at 2.4 GHz. (Sizing it larger to cover
                # the later x-block gaps loses: that window is DMA-
                # throughput-bound, so extra dummy work only delays the
                # stream.)
                nc.vector.memset(warm_s[:], 0.0)
                warm_p = psum1.tile([128, 320], F32, tag="h1p")
                for i in range(KD):
                    nc.tensor.matmul(warm_p[:], warm_s[:, 0:128],
                                     warm_s[:, 128:448],
                                     start=(i == 0), stop=(i == KD - 1))

                def swiglu_block(w1c, w3c, x_t, h_s, f, g0, tn):
                    # x_t: per-block x tile (read at local offset 0);
                    # g0: global token offset for the h-slab write
                    h1p = psum1.tile([128, tn], F32, tag="h1p")
                    h3p = psum1.tile([128, tn], F32, tag="h3p", bufs=2)
                    for k in range(KD):
                        nc.tensor.matmul(
                            h1p[:], w1c[:, k, :], x_t[:, k, 0:tn],
                            start=(k == 0), stop=(k == KD - 1))
                    for k in range(KD):
                        nc.tensor.matmul(
                            h3p[:], w3c[:, k, :], x_t[:, k, 0:tn],
                            start=(k == 0), stop=(k == KD - 1))
                    s1 = spool.tile([128, tn], F32, tag="s1")
                    nc.scalar.activation(s1[:], h1p[:], silu)
                    nc.vector.tensor_mul(h_s[:, f, g0:g0 + tn], s1[:], h3p[:])

                # f0's tail blocks (j>=2) are gated on the deepest x DMA
                # (xb2, ~25us into the serialized transfer queue). Defer
                # them until after f1's b0/b1 — list-scheduling against
                # DMA availability — so the PE consumes ready work while
                # xb2 streams in.
                defer_tail = KF > 1 and len(xblocks) > 2
                f0_w = None
                for f in range(KF):
                    # side first everywhere: its x/weights are first in
                    # the DMA queues, and at f=KF-1 its hs slab completes
                    # early so GEMM2 starts with zero boundary gap.
                    # w1 streams on the SP ring, w3 on the gpsimd SWDGE —
                    # two rings so issue overhead doesn't serialize.
                    if cap_s:
                        w1sc = wpool.tile([128, KD, 128], F16, tag="w1s")
                        nc.sync.dma_start(w1sc[:], w1s_d[f])
                        w3sc = wpool.tile([128, KD, 128], F16, tag="w3s")
                        nc.gpsimd.dma_start(w3sc[:], w3s_d[f])
                    w1mc = wpool.tile([128, KD, 128], F16, tag="w1m")
                    nc.sync.dma_start(w1mc[:], w1m_d[f])
                    w3mc = wpool.tile([128, KD, 128], F16, tag="w3m")
                    nc.gpsimd.dma_start(w3mc[:], w3m_d[f])
                    if f == KF - 2:
                        # m=0 GEMM2 weights prefetch on the now-idle ACT
                        # ring, with DMA-bus slack (startup is long past)
                        nc.scalar.dma_start(w2m0[:], w2m_d[0])
                        if cap_s:
                            nc.scalar.dma_start(w2s0[:], w2s_d[0])

                    if cap_s:
                        swiglu_block(w1sc, w3sc, xs_s, hs_s, f, 0, cap_s)
                    n_now = len(xblocks)
                    if defer_tail and f == 0:
                        n_now = 2
                        f0_w = (w1mc, w3mc)
                    for j in range(n_now):
                        b0, bw = xblocks[j]
                        swiglu_block(w1mc, w3mc, xb_s[j], hm_s, f, b0, bw)
                    if defer_tail and f == 1:
                        for j in range(2, len(xblocks)):
                            b0, bw = xblocks[j]
                            swiglu_block(f0_w[0], f0_w[1], xb_s[j],
                                         hm_s, 0, b0, bw)

            # ---- Phase 2: yT = w2 @ hT ----
            with (
                tc.tile_pool(name="w2", bufs=2) as w2pool,
                tc.tile_pool(name="yo", bufs=3) as ypool,
            ):
                def out_block(w2c, h_s, y_d, m, t0, tn, last=False):
                    yp = psum1.tile([128, tn], F32, tag="yp")
                    for k2 in range(KF):
                        nc.tensor.matmul(
                            yp[:], w2c[:, k2, :], h_s[:, k2, t0:t0 + tn],
                            start=(k2 == 0), stop=(k2 == KF - 1))
                    yo = ypool.tile([128, tn], F16, tag="yo")
                    nc.vector.tensor_copy(yo[:], yp[:])
                    # the very last store rides the long-idle SP ring so its
                    # issue doesn't queue behind the m=KD-1 main stores on
                    # ACT (shortens the end-of-kernel drain)
                    ring = nc.sync if last else nc.scalar
                    ring.dma_start(y_d[m][:, t0:t0 + tn], yo[:])

                for m in range(KD):
                    if m == 0:
                        w2mc, w2sc = w2m0, (w2s0 if cap_s else None)
                    else:
                        w2mc = w2pool.tile([128, KF, 128], F16, tag="w2m")
                        nc.sync.dma_start(w2mc[:], w2m_d[m])
                        if cap_s:
                            w2sc = w2pool.tile([128, KF, 128], F16, tag="w2s")
                            nc.sync.dma_start(w2sc[:], w2s_d[m])
                    # m=0: side first (its hs slab is complete earliest, so
                    # GEMM2 starts with no boundary gap); else side last so
                    # the final drain is the tiny side tile.
                    if cap_s and m == 0:
                        out_block(w2sc, hs_s, ys_d, m, 0, cap_s)
                    for i, (t0, tn) in enumerate(tiles_m):
                        out_block(w2mc, hm_s, ym_d, m, t0, tn,
                                  last=(m == KD - 1 and not cap_s
                                        and i == len(tiles_m) - 1))
                    if cap_s and m > 0:
                        out_block(w2sc, hs_s, ys_d, m, 0, cap_s,
                                  last=(m == KD - 1))

    nc.compile()
    return nc


def _plan(counts):
    """Pick (cap_m, cap_s): every core computes cap_m tokens of its own
    expert plus one cap_s-token side block of an overflowing expert.
    Minimizes cap_m + cap_s subject to total overflow chunks <= N_CORES."""
    mx = int(max(counts))
    cap0 = max(512, -(-mx // 16) * 16)      # no-side fallback
    best = (cap0, cap0, 0)                  # (cost, cap_m, cap_s)
    for s in range(24, 132, 4):
        lo = max(512, mx - s * N_CORES)
        for cap_m in range(-(-lo // 2) * 2, mx + 1, 2):
            need = sum(-(-max(0, int(n) - cap_m) // s) for n in counts)
            if need <= N_CORES:
                cost = cap_m + s
                if cost < best[0] or (cost == best[0]
                                      and abs(s - 64) < abs(best[2] - 64)):
                    best = (cost, cap_m, s)
                break
    _, cap_m, cap_s = best
    return (cap_m, cap_s) if cap_s and cap_m + cap_s < cap0 else (cap0, 0)


def kernel(x, expert_indices, expert_weights, w1, w2, w3):
    global LAST_EXEC_NS
    x = np.ascontiguousarray(np.asarray(x, dtype=np.float32))
    routing = np.asarray(expert_indices)
    probs = np.asarray(expert_weights, dtype=np.float32)
    w1 = np.asarray(w1, dtype=np.float32)
    w2 = np.asarray(w2, dtype=np.float32)
    w3 = np.asarray(w3, dtype=np.float32)
    n_tokens = x.shape[0]

    idxs = [np.flatnonzero(routing[:, e]) for e in range(NUM_EXPERTS)]
    counts = [len(i) for i in idxs]
    cap_m, cap_s = _plan(counts)

    # Assign overflow chunks (expert, start, count) to the 8 side slots
    slots = []
    if cap_s:
        for e in range(NUM_EXPERTS):
            off = cap_m
            while off < counts[e]:
                cnt = min(cap_s, counts[e] - off)
                slots.append((e, off, cnt))
                off += cnt
        assert len(slots) <= N_CORES, (cap_m, cap_s, counts)
    slots += [None] * (N_CORES - len(slots))

    if (cap_m, cap_s) not in _PROGRAMS:
        _PROGRAMS[(cap_m, cap_s)] = _build_program(cap_m, cap_s)
    nc = _PROGRAMS[(cap_m, cap_s)]

    x16 = x.astype(np.float16)
    w1_16 = w1.astype(np.float16)
    w3_16 = w3.astype(np.float16)
    w2_16 = w2.astype(np.float16)

    def _wprep(e):
        # W1T[f,p,k,m] = w1[e][f*128+m, k*128+p]; W2T[m,p,k2,d] = w2[e][m*128+d, k2*128+p]
        return (
            np.ascontiguousarray(
                w1_16[e].reshape(KF, 128, KD, 128).transpose(0, 3, 2, 1)),
            np.ascontiguousarray(
                w3_16[e].reshape(KF, 128, KD, 128).transpose(0, 3, 2, 1)),
            np.ascontiguousarray(
                w2_16[e].reshape(KD, 128, KF, 128).transpose(0, 3, 2, 1)),
        )

    from concurrent.futures import ThreadPoolExecutor
    with ThreadPoolExecutor(max_workers=NUM_EXPERTS) as pool:
        wt = list(pool.map(_wprep, range(NUM_EXPERTS)))

    def _xgather(idx, cap):
        # [128, KD, cap] partition-major: out[p, k, t] = x[idx[t], k*128+p]
        out = np.zeros((128, KD, cap), dtype=np.float16)
        if len(idx):
            out[:, :, :len(idx)] = (
                x16[idx].T.reshape(KD, 128, len(idx)).transpose(1, 0, 2))
        return out

    xblocks = _xblocks(cap_m)
    zero_w = None
    in_maps = []
    for c in range(N_CORES):
        xmh = _xgather(idxs[c][:cap_m], cap_m)
        m = {"w1m": wt[c][0], "w3m": wt[c][1], "w2m": wt[c][2]}
        for j, (b0, bw) in enumerate(xblocks):
            m[f"xm{j}"] = np.ascontiguousarray(xmh[:, :, b0:b0 + bw])
        if cap_s:
            if slots[c] is not None:
                e, off, cnt = slots[c]
                m["xs"] = _xgather(idxs[e][off:off + cnt], cap_s)
                m["w1s"], m["w3s"], m["w2s"] = wt[e]
            else:
                if zero_w is None:
                    zero_w = (
                        np.zeros((128, KD, cap_s), np.float16),
                        np.zeros((KF, 128, KD, 128), np.float16),
                        np.zeros((KD, 128, KF, 128), np.float16),
                    )
                m["xs"] = zero_w[0]
                m["w1s"] = m["w3s"] = zero_w[1]
                m["w2s"] = zero_w[2]
        in_maps.append(m)

    trace = os.environ.get("BASS_KERNEL_TRACE", "0") == "1"
    if trace:
        import importlib.util
        if importlib.util.find_spec("antenv") is None or importlib.util.find_spec(
                "antenv.axon_hooks") is None:
            trace = False  # NTFF hook unavailable in this environment
    res = run_bass_kernel_spmd(
        nc, in_maps, core_ids=list(range(N_CORES)),
        trace=trace, trace_cores=list(range(N_CORES)) if trace else None,
    )
    LAST_EXEC_NS = res.exec_time_ns

    out = np.zeros((n_tokens, DIM), dtype=np.float32)
    for e in range(NUM_EXPERTS):
        idx = idxs[e][:cap_m]
        y_t = res.results[e]["ym"].reshape(DIM, cap_m)[:, :len(idx)]
        out[idx] += probs[idx, e][:, None] * y_t.T.astype(np.float32)
    for c in range(N_CORES):
        if cap_s and slots[c] is not None:
            e, off, cnt = slots[c]
            idx = idxs[e][off:off + cnt]
            y_t = res.results[c]["ys"].reshape(DIM, cap_s)[:, :cnt]
            out[idx] += probs[idx, e][:, None] * y_t.T.astype(np.float32)
    return out
